# revision 38
# baseline (speedup 1.0000x reference)
import os
import sys
import types
from contextlib import ExitStack

sys.path.insert(0, "/opt/trn_rl_repo")

import numpy as np
import ml_dtypes
from ml_dtypes import bfloat16 as np_bf16

import concourse.bacc as bacc
import concourse.tile as tile
import concourse.mybir as mybir
from concourse import bass_utils, masks
from concourse.bass_utils import run_bass_kernel_spmd

NCORES = 8
B, N, HX, HS = 32, 4096, 128, 1024
F = 512            # HX * R
COLS = 16384       # W columns per core
NB = 32            # 512-col param blocks per core
NG = 8             # 4-block groups (16KB per partition per DMA)
SPC = B // NCORES  # samples per core
TS = 512           # tokens per block
TB = N // TS
WSCALE = 64.0      # host-side scale applied to W before fp8 quantization
CT = NB * F        # 16384 param cols per core

LAST_EXEC_NS = None
_cached_nc = None


def _ensure_axon_hooks():
    try:
        import antenv.axon_hooks  # noqa: F401
        return
    except Exception:
        pass
    hook = None
    try:
        import trn_agent_boot.trn_boot as tb
        hook = tb._ntff_profile_via_ctypes("/opt/axon/libaxon_pjrt.so")
    except Exception:
        hook = None
    mod = types.ModuleType("antenv.axon_hooks")
    mod.get_axon_ntff_profile_hook = lambda: hook
    sys.modules["antenv.axon_hooks"] = mod
    try:
        bass_utils.upload_artifacts = lambda tmpdir: tmpdir
    except Exception:
        pass


def _build():
    fp32 = mybir.dt.float32
    bf16 = mybir.dt.bfloat16
    fp8 = mybir.dt.float8e3
    AF = mybir.ActivationFunctionType
    ALU = mybir.AluOpType

    nc = bacc.Bacc("TRN2", target_bir_lowering=False, debug=False,
                   num_devices=NCORES)
    W_d = nc.dram_tensor("W", [NG, 128, 4 * 8 * F], fp8, kind="ExternalInput")
    x_d = nc.dram_tensor("x", [SPC, HX, N], bf16, kind="ExternalInput")
    s_d = nc.dram_tensor("s", [128, 8 * B], bf16, kind="ExternalInput")
    b1_d = nc.dram_tensor("b1", [HX, F], bf16, kind="ExternalInput")
    b2_d = nc.dram_tensor("b2", [128, 4 * HX], bf16, kind="ExternalInput")
    g_d = nc.dram_tensor("g", [HX, 1], fp32, kind="ExternalInput")
    o_d = nc.dram_tensor("o", [SPC, HX, N], bf16, kind="ExternalOutput")

    with tile.TileContext(nc) as tc:
        with tc.tile_pool(name="pers", bufs=1) as pers, \
             tc.tile_pool(name="xres", bufs=1) as xres, \
             tc.tile_pool(name="dram", bufs=1, space="DRAM") as dram:
            s_t = pers.tile([128, 8 * B], bf16)
            nc.sync.dma_start(s_t[:], s_d[:])
            g_t = pers.tile([HX, 1], fp32)
            nc.sync.dma_start(g_t[:], g_d[:])
            b1_t = pers.tile([HX, F], bf16)
            nc.scalar.dma_start(b1_t[:], b1_d[:])
            b2_t = pers.tile([128, 4 * HX], bf16)
            nc.scalar.dma_start(b2_t[:], b2_d[:])
            ones_col = pers.tile([128, 1], bf16)
            nc.vector.memset(ones_col[:], 1.0)
            ones_row = pers.tile([1, 128], bf16)
            nc.vector.memset(ones_row[:], 1.0)
            eps_t = pers.tile([128, 1], fp32)
            nc.vector.memset(eps_t[:], 1e-6)
            ident = pers.tile([128, 128], fp32)
            masks.make_identity(nc, ident[:])
            zeroB = pers.tile([B, 4 * F], bf16)
            nc.vector.memset(zeroB[:], 0.0)

            in_all = dram.tile([B, CT], bf16, name="in_all")
            out_all = dram.tile([B, CT], bf16, name="out_all")

            st_all = pers.tile([B, CT], bf16, name="st_all")

            xts = [xres.tile([HX, N], bf16, name=f"xt{i}")
                   for i in range(SPC)]

            # ---- phase A: params = s @ (64*W) / 64, streamed in 8 groups
            # of 4 blocks (16KB per partition per DMA descriptor).  W rides
            # as fp8e3 (half the HBM bytes of bf16); the 1/64 descale is in
            # the PSUM->SBUF copy.
            with tc.tile_pool(name="wp", bufs=3) as wp, \
                 tc.tile_pool(name="psA", bufs=2, space="PSUM") as psA:
                for g in range(NG):
                    wt = wp.tile([128, 4 * 8 * F], fp8)
                    weng = nc.sync if g % 2 == 0 else nc.scalar
                    nh = 8 if g == 0 else 2
                    for i in range(nh):
                        w = 4 * 8 * F // nh
                        weng.dma_start(wt[:, i * w:(i + 1) * w],
                                       W_d[g, :, i * w:(i + 1) * w])
                    psg = psA.tile([B, 4 * F], fp32)
                    for i in range(4):
                        for kt in range(8):
                            nc.tensor.matmul(
                                psg[:, i * F:(i + 1) * F],
                                s_t[:, kt * B:(kt + 1) * B],
                                wt[:, i * 8 * F + kt * F:
                                   i * 8 * F + (kt + 1) * F],
                                start=(kt == 0), stop=(kt == 7),
                            )
                    sl = slice(g * 4 * F, (g + 1) * 4 * F)
                    if g % 2 == 0 or g == NG - 1:
                        # keep the collective-gating descale on scalar
                        # (in-order engine; DVE reorders by readiness)
                        nc.scalar.activation(st_all[:, sl], psg[:], AF.Copy,
                                             scale=1.0 / WSCALE)
                    else:
                        # DVE is idle in phase A; halving the scalar load
                        # frees it earlier for the window chain
                        nc.vector.scalar_tensor_tensor(
                            st_all[:, sl], psg[:], 1.0 / WSCALE, zeroB[:],
                            ALU.mult, ALU.add)
                    nc.gpsimd.dma_start(in_all[:, sl], st_all[:, sl])
                    if g in (4, 5):
                        # x rides mid-stream so it lands before the
                        # collective payload can delay it
                        for i in (0, 2) if g == 4 else (1, 3):
                            weng.dma_start(xts[i][:], x_d[i, :, :])

            # ---- one all-to-all for the full 1MB of params
            nc.gpsimd.collective_compute(
                "AllToAll", ALU.bypass,
                replica_groups=[list(range(NCORES))],
                ins=[in_all[:].opt()], outs=[out_all[:].opt()],
            )

            # ---- collective window: rmsnorm stats + normalized x tiles
            rr_flats = []
            with tc.tile_pool(name="xsqp", bufs=1) as xsqp, \
                 tc.tile_pool(name="stm", bufs=2) as stm, \
                 tc.tile_pool(name="psS", bufs=2, space="PSUM") as psS:
                for i in range(SPC):
                    xt = xts[i]
                    xsq = xsqp.tile([HX, N], bf16)
                    for ch in range(2):
                        sl = slice(ch * (N // 2), (ch + 1) * (N // 2))
                        nc.vector.tensor_tensor(xsq[:, sl], xt[:, sl],
                                                xt[:, sl], ALU.mult)
                    pn_s = psS.tile([128, 32], fp32, name="pn_s")
                    for c in range(32):
                        nc.tensor.matmul(
                            pn_s[:, c:c + 1],
                            xsq[:, c * 128:(c + 1) * 128],
                            ones_col[:],
                            start=True, stop=True,
                        )
                    sq_m = stm.tile([128, 32], fp32, name="sq_m")
                    nc.scalar.activation(sq_m[:], pn_s[:], AF.Sqrt,
                                         bias=eps_t[:], scale=1.0 / HX)
                    rr = stm.tile([128, 32], fp32, name="rr")
                    nc.vector.reciprocal(rr[:], sq_m[:])
                    rr_t = psS.tile([32, 128], fp32, name="rr_t")
                    nc.tensor.transpose(rr_t[:], rr[:], ident[:])
                    rr_ts = stm.tile([32, 128], bf16, name="rr_ts")
                    nc.vector.tensor_copy(rr_ts[:], rr_t[:])
                    rr_flat = pers.tile([1, N], bf16, name=f"rr_flat{i}")
                    nc.scalar.dma_start(rr_flat[:], rr_ts[:])
                    rr_flats.append(rr_flat)

            xss = []
            for i in range(SPC):
                xs = xres.tile([HX, N], bf16, name=f"xs{i}")
                xss.append(xs)
            with tc.tile_pool(name="psB", bufs=2, space="PSUM") as psB, \
                 tc.tile_pool(name="rrs", bufs=2) as rrs:
                for i in range(SPC):
                    for h in range(2):
                        sl = slice(h * (N // 2), (h + 1) * (N // 2))
                        rrb = psB.tile([128, N // 2], fp32, name="rrb")
                        for q in range(4):
                            nc.tensor.matmul(
                                rrb[:, q * TS:(q + 1) * TS], ones_row[:],
                                rr_flats[i][0:1,
                                            h * (N // 2) + q * TS:
                                            h * (N // 2) + (q + 1) * TS],
                                start=True, stop=True)
                        if i < 2:
                            # scalar is idle in the window: cast first, so
                            # the DVE multiply runs at bf16 speed
                            rrc = rrs.tile([128, N // 2], bf16, name="rrc")
                            nc.scalar.activation(rrc[:], rrb[:], AF.Copy)
                            nc.vector.tensor_tensor(xss[i][:, sl],
                                                    xts[i][:, sl],
                                                    rrc[:], ALU.mult)
                        else:
                            nc.vector.tensor_tensor(xss[i][:, sl],
                                                    xts[i][:, sl],
                                                    rrb[:], ALU.mult)

            # ---- post-collective: gather per-sample weights, bias, norms
            with ExitStack() as es:
                def pool(name, bufs, space=None):
                    kw = {"space": space} if space else {}
                    return es.enter_context(
                        tc.tile_pool(name=name, bufs=bufs, **kw))
                p_fc1 = pool("fc1", 1)
                p_fc2 = pool("fc2", 1)
                p_fc1g = pool("fc1g", 1)
                p_rn = pool("rn", 1)
                p_tmp = pool("tmp", 2)
                p_sq = pool("sq", 2)
                p_h1 = pool("h1", 2)
                p_ob = pool("ob", 2)

                oa_sj = out_all[:, :].rearrange("(s j) c -> s j c", s=8)

                es_ps = ExitStack()
                p_pp = es_ps.enter_context(
                    tc.tile_pool(name="pp", bufs=1, space="PSUM"))
                p_rnb = es_ps.enter_context(
                    tc.tile_pool(name="rnb", bufs=2, space="PSUM"))

                # gathers: fc1 rows (sync ring), fc2 cols (scalar+gpsimd)
                fc1rs, fc2cs = [], []
                for j in range(SPC):
                    fc1r = p_fc1.tile([HX, F], bf16, name=f"fc1r{j}")
                    for sc in range(4):
                        nc.sync.dma_start(
                            fc1r[32 * sc:32 * (sc + 1), :],
                            oa_sj[sc, j, :].rearrange("(a f) -> a f", a=32),
                        )
                    fc2c = p_fc2.tile([128, 4 * HX], bf16, name=f"fc2c{j}")
                    for fb in range(4):
                        eng = nc.scalar if fb % 2 == 0 else nc.gpsimd
                        eng.dma_start(
                            fc2c[:, fb * HX:(fb + 1) * HX],
                            oa_sj[4 + fb, j, :].rearrange("(p d) -> p d",
                                                          p=128),
                        )
                    fc1rs.append(fc1r)
                    fc2cs.append(fc2c)

                # per-sample: bias, squares, norms, rsqrt, rank-1 fold.
                # Sample-major pnn columns (j*5 + {0..3} fc1 chunks, j*5+4
                # fc2) so each sample's chain completes independently.
                pnn = p_pp.tile([128, 20], fp32, name="pnn")
                fc1bs, fc2bs, fc1gs = [], [], []
                rn_sb = p_rn.tile([128, 20], fp32, name="rn_sb")
                for j in range(SPC):
                    fc1b = p_fc1.tile([HX, F], bf16, name=f"fc1b{j}")
                    nc.vector.tensor_tensor(fc1b[:], fc1rs[j][:], b1_t[:],
                                            ALU.add)
                    sq1 = p_sq.tile([HX, F], bf16, name="sq1")
                    nc.vector.tensor_tensor(sq1[:], fc1b[:], fc1b[:],
                                            ALU.mult)
                    for jc in range(4):
                        nc.tensor.matmul(
                            pnn[:, j * 5 + jc:j * 5 + jc + 1],
                            sq1[:, jc * 128:(jc + 1) * 128],
                            ones_col[:], start=True, stop=True)
                    fc2b = p_fc2.tile([128, 4 * HX], bf16, name=f"fc2b{j}")
                    nc.vector.tensor_tensor(fc2b[:], fc2cs[j][:], b2_t[:],
                                            ALU.add)
                    sq2 = p_sq.tile([128, 4 * HX], bf16, name="sq2")
                    nc.vector.tensor_tensor(sq2[:], fc2b[:], fc2b[:],
                                            ALU.mult)
                    for fb in range(4):
                        nc.tensor.matmul(
                            pnn[:, j * 5 + 4:j * 5 + 5],
                            sq2[:, fb * 128:(fb + 1) * 128],
                            ones_col[:], start=(fb == 0), stop=(fb == 3))
                    nn = p_tmp.tile([128, 5], fp32, name="nn")
                    nc.scalar.activation(nn[:], pnn[:, j * 5:j * 5 + 5],
                                         AF.Sqrt)
                    nc.vector.reciprocal(rn_sb[:, j * 5:j * 5 + 5], nn[:])
                    trn = p_pp.tile([4, 128], fp32, name="trn",
                                    uniquify=True)
                    nc.tensor.transpose(trn[:],
                                        rn_sb[:, j * 5:j * 5 + 4], ident[:])
                    rn_t = p_tmp.tile([4, 128], bf16, name="rn_t")
                    nc.vector.tensor_copy(rn_t[:], trn[:])
                    rn1f = p_rn.tile([1, F], bf16, name=f"rn1f{j}")
                    nc.gpsimd.dma_start(rn1f[:], rn_t[:])
                    rn1b = p_rnb.tile([HX, F], fp32, name="rn1b")
                    nc.tensor.matmul(rn1b[:], ones_row[:], rn1f[:],
                                     start=True, stop=True)
                    fc1g = p_fc1g.tile([HX, F], bf16, name=f"fc1g{j}")
                    nc.vector.scalar_tensor_tensor(
                        fc1g[:], fc1b[:], g_t[:], rn1b[:],
                        ALU.mult, ALU.mult)
                    fc1bs.append(fc1b)
                    fc2bs.append(fc2b)
                    fc1gs.append(fc1g)
                es_ps.close()

                # ---- main loop: software-pipelined (sample, token-block)
                with tc.tile_pool(name="ph1a", bufs=2,
                                  space="PSUM") as p_ph1a, \
                     tc.tile_pool(name="ph1b", bufs=1,
                                  space="PSUM") as p_ph1b, \
                     tc.tile_pool(name="ph2", bufs=2, space="PSUM") as p_ph2:
                    units = [(j, tb) for j in range(SPC) for tb in range(TB)]
                    prev = None

                    def bmm2_first(pv):
                        nc.tensor.matmul(pv["ph2"][:],
                                         fc2bs[pv["j"]][:, 0:HX],
                                         pv["h1a"][:, 0:TS],
                                         start=True, stop=False)
                        nc.tensor.matmul(pv["ph2"][:],
                                         fc2bs[pv["j"]][:, HX:2 * HX],
                                         pv["h1a"][:, TS:2 * TS],
                                         start=False, stop=False)

                    def bmm2_second(pv):
                        nc.tensor.matmul(pv["ph2"][:],
                                         fc2bs[pv["j"]][:, 2 * HX:3 * HX],
                                         pv["h1b"][:, 0:TS],
                                         start=False, stop=False)
                        nc.tensor.matmul(pv["ph2"][:],
                                         fc2bs[pv["j"]][:, 3 * HX:4 * HX],
                                         pv["h1b"][:, TS:2 * TS],
                                         start=False, stop=True)
                        ob = p_ob.tile([HX, TS], bf16)
                        nc.vector.scalar_tensor_tensor(
                            ob[:], pv["ph2"][:],
                            rn_sb[:, 5 * pv["j"] + 4:5 * pv["j"] + 5],
                            pv["xv"],
                            ALU.mult, ALU.add)
                        nc.sync.dma_start(
                            o_d[pv["j"], :,
                                pv["tb"] * TS:(pv["tb"] + 1) * TS],
                            ob[:])

                    for j, tb in units:
                        xv = xts[j][:, tb * TS:(tb + 1) * TS]
                        xsv = xss[j][:, tb * TS:(tb + 1) * TS]
                        fc1g = fc1gs[j]

                        ph1a = p_ph1a.tile([128, 2 * TS], fp32)
                        nc.tensor.matmul(ph1a[:, 0:TS], fc1g[:, 0:128], xsv,
                                         start=True, stop=True)
                        nc.tensor.matmul(ph1a[:, TS:2 * TS],
                                         fc1g[:, 128:256], xsv,
                                         start=True, stop=True)
                        h1a = p_h1.tile([128, 2 * TS], bf16)
                        nc.scalar.activation(h1a[:], ph1a[:], AF.Silu)

                        if prev is not None:
                            prev["ph2"] = p_ph2.tile([HX, TS], fp32,
                                                     name="ph2")
                            bmm2_first(prev)

                        ph1b = p_ph1b.tile([128, 2 * TS], fp32)
                        nc.tensor.matmul(ph1b[:, 0:TS], fc1g[:, 256:384],
                                         xsv, start=True, stop=True)
                        nc.tensor.matmul(ph1b[:, TS:2 * TS],
                                         fc1g[:, 384:512], xsv,
                                         start=True, stop=True)
                        h1b = p_h1.tile([128, 2 * TS], bf16)
                        nc.scalar.activation(h1b[:], ph1b[:], AF.Silu)

                        if prev is not None:
                            bmm2_second(prev)

                        prev = {"j": j, "tb": tb, "xv": xv,
                                "h1a": h1a, "h1b": h1b}

                    prev["ph2"] = p_ph2.tile([HX, TS], fp32, name="ph2")
                    bmm2_first(prev)
                    bmm2_second(prev)
    nc.compile()
    return nc


def _prep_inputs(x, s, W, b, g):
    e3m4 = ml_dtypes.float8_e3m4
    s_p = np.ascontiguousarray(
        s.T.reshape(8, 128, B).transpose(1, 0, 2).reshape(128, 8 * B)
    ).astype(np_bf16)
    g_p = np.ascontiguousarray(g.reshape(HX, 1)).astype(np.float32)
    b1 = np.ascontiguousarray(b[:HX * F].reshape(HX, F)).astype(np_bf16)
    b2 = np.ascontiguousarray(
        b[HX * F:].reshape(4, 128, 128).transpose(1, 0, 2).reshape(128, 512)
    ).astype(np_bf16)
    W8 = np.clip(W * WSCALE, -15.5, 15.5).astype(e3m4)
    in_maps = []
    for c in range(NCORES):
        Wc = W8[:, c * COLS:(c + 1) * COLS]
        # [HS, COLS] -> [NB, 128, 8*F] (block, partition, kt*F+j)
        Wc = np.ascontiguousarray(
            Wc.reshape(8, 128, NB, F).transpose(2, 1, 0, 3)
              .reshape(NB, 128, 8 * F))
        # group 4 consecutive blocks per partition row (16KB descriptors)
        Wc = np.ascontiguousarray(
            Wc.reshape(NG, 4, 128, 8 * F).transpose(0, 2, 1, 3)
              .reshape(NG, 128, 4 * 8 * F))
        xc = np.ascontiguousarray(
            x[SPC * c:SPC * (c + 1)].transpose(0, 2, 1)).astype(np_bf16)
        in_maps.append({"W": Wc, "x": xc, "s": s_p, "b1": b1, "b2": b2,
                        "g": g_p})
    return in_maps


def kernel(x, s, W, b, g):
    global LAST_EXEC_NS, _cached_nc
    x = np.asarray(x, dtype=np.float32)
    s = np.asarray(s, dtype=np.float32)
    W = np.asarray(W, dtype=np.float32)
    b = np.asarray(b, dtype=np.float32)
    g = np.asarray(g, dtype=np.float32)

    trace = os.environ.get("KERNEL_TRACE", "0") == "1"
    if trace:
        _ensure_axon_hooks()
    if _cached_nc is None:
        _cached_nc = _build()
    in_maps = _prep_inputs(x, s, W, b, g)
    res = run_bass_kernel_spmd(_cached_nc, in_maps, list(range(NCORES)),
                               trace=trace)
    LAST_EXEC_NS = res.exec_time_ns
    out = np.concatenate([res.results[c]["o"] for c in range(NCORES)], axis=0)
    return np.ascontiguousarray(
        out.transpose(0, 2, 1).astype(np.float32))


# revision 39
# speedup vs baseline: 1.0633x; 1.0633x over previous
import os
import sys
import types
from contextlib import ExitStack

sys.path.insert(0, "/opt/trn_rl_repo")

import numpy as np
import ml_dtypes
from ml_dtypes import bfloat16 as np_bf16

import concourse.bacc as bacc
import concourse.tile as tile
import concourse.mybir as mybir
from concourse import bass_utils, masks
from concourse.bass_utils import run_bass_kernel_spmd

NCORES = 8
B, N, HX, HS = 32, 4096, 128, 1024
F = 512            # HX * R
COLS = 16384       # W columns per core
NB = 32            # 512-col param blocks per core
NG = 8             # 4-block groups (16KB per partition per DMA)
SPC = B // NCORES  # samples per core
TS = 512           # tokens per block
TB = N // TS
WSCALE = 64.0      # host-side scale applied to W before fp8 quantization
CT = NB * F        # 16384 param cols per core

LAST_EXEC_NS = None
_cached_nc = None


def _ensure_axon_hooks():
    try:
        import antenv.axon_hooks  # noqa: F401
        return
    except Exception:
        pass
    hook = None
    try:
        import trn_agent_boot.trn_boot as tb
        hook = tb._ntff_profile_via_ctypes("/opt/axon/libaxon_pjrt.so")
    except Exception:
        hook = None
    mod = types.ModuleType("antenv.axon_hooks")
    mod.get_axon_ntff_profile_hook = lambda: hook
    sys.modules["antenv.axon_hooks"] = mod
    try:
        bass_utils.upload_artifacts = lambda tmpdir: tmpdir
    except Exception:
        pass


def _build():
    fp32 = mybir.dt.float32
    bf16 = mybir.dt.bfloat16
    fp8 = mybir.dt.float8e3
    AF = mybir.ActivationFunctionType
    ALU = mybir.AluOpType

    nc = bacc.Bacc("TRN2", target_bir_lowering=False, debug=False,
                   num_devices=NCORES)
    W_d = nc.dram_tensor("W", [NG, 128, 4 * 8 * F], fp8, kind="ExternalInput")
    x_d = nc.dram_tensor("x", [SPC, HX, N], bf16, kind="ExternalInput")
    s_d = nc.dram_tensor("s", [128, 8 * B], bf16, kind="ExternalInput")
    b1_d = nc.dram_tensor("b1", [HX, F], bf16, kind="ExternalInput")
    b2_d = nc.dram_tensor("b2", [128, 4 * HX], bf16, kind="ExternalInput")
    g_d = nc.dram_tensor("g", [HX, 1], fp32, kind="ExternalInput")
    o_d = nc.dram_tensor("o", [SPC, HX, N], bf16, kind="ExternalOutput")

    with tile.TileContext(nc) as tc:
        with tc.tile_pool(name="pers", bufs=1) as pers, \
             tc.tile_pool(name="xres", bufs=1) as xres, \
             tc.tile_pool(name="dram", bufs=1, space="DRAM") as dram:
            s_t = pers.tile([128, 8 * B], bf16)
            nc.sync.dma_start(s_t[:], s_d[:])
            g_t = pers.tile([HX, 1], fp32)
            nc.sync.dma_start(g_t[:], g_d[:])
            b1_t = pers.tile([HX, F], bf16)
            nc.scalar.dma_start(b1_t[:], b1_d[:])
            b2_t = pers.tile([128, 4 * HX], bf16)
            nc.scalar.dma_start(b2_t[:], b2_d[:])
            ones_col = pers.tile([128, 1], bf16)
            nc.vector.memset(ones_col[:], 1.0)
            ones_row = pers.tile([1, 128], bf16)
            nc.vector.memset(ones_row[:], 1.0)
            eps_t = pers.tile([128, 1], fp32)
            nc.vector.memset(eps_t[:], 1e-6)
            ident = pers.tile([128, 128], fp32)
            masks.make_identity(nc, ident[:])
            zeroB = pers.tile([B, 4 * F], bf16)
            nc.vector.memset(zeroB[:], 0.0)

            # params ride the wire as fp8e3 (~1.4% end-to-end err, sim-
            # verified): halves the collective payload, the CC-gating
            # in_all writes, and the post-collective gather bytes
            in_all = dram.tile([B, CT], fp8, name="in_all")
            out_all = dram.tile([B, CT], fp8, name="out_all")

            st_all = pers.tile([B, CT], fp8, name="st_all")

            xts = [xres.tile([HX, N], bf16, name=f"xt{i}")
                   for i in range(SPC)]

            # ---- phase A: params = s @ (64*W) / 64, streamed in 8 groups
            # of 4 blocks (16KB per partition per DMA descriptor).  W rides
            # as fp8e3 (half the HBM bytes of bf16); the 1/64 descale is in
            # the PSUM->SBUF copy.
            with tc.tile_pool(name="wp", bufs=3) as wp, \
                 tc.tile_pool(name="psA", bufs=2, space="PSUM") as psA:
                for g in range(NG):
                    wt = wp.tile([128, 4 * 8 * F], fp8)
                    weng = nc.sync if g % 2 == 0 else nc.scalar
                    nh = 8 if g == 0 else 2
                    for i in range(nh):
                        w = 4 * 8 * F // nh
                        weng.dma_start(wt[:, i * w:(i + 1) * w],
                                       W_d[g, :, i * w:(i + 1) * w])
                    psg = psA.tile([B, 4 * F], fp32)
                    for i in range(4):
                        for kt in range(8):
                            nc.tensor.matmul(
                                psg[:, i * F:(i + 1) * F],
                                s_t[:, kt * B:(kt + 1) * B],
                                wt[:, i * 8 * F + kt * F:
                                   i * 8 * F + (kt + 1) * F],
                                start=(kt == 0), stop=(kt == 7),
                            )
                    sl = slice(g * 4 * F, (g + 1) * 4 * F)
                    if g % 2 == 0 or g == NG - 1:
                        # keep the collective-gating descale on scalar
                        # (in-order engine; DVE reorders by readiness)
                        nc.scalar.activation(st_all[:, sl], psg[:], AF.Copy,
                                             scale=1.0 / WSCALE)
                    else:
                        # DVE is idle in phase A; halving the scalar load
                        # frees it earlier for the window chain
                        nc.vector.scalar_tensor_tensor(
                            st_all[:, sl], psg[:], 1.0 / WSCALE, zeroB[:],
                            ALU.mult, ALU.add)
                    nc.gpsimd.dma_start(in_all[:, sl], st_all[:, sl])
                    if g in (4, 5):
                        # x rides mid-stream so it lands before the
                        # collective payload can delay it
                        for i in (0, 2) if g == 4 else (1, 3):
                            weng.dma_start(xts[i][:], x_d[i, :, :])

            # ---- one all-to-all for the full 1MB of params
            nc.gpsimd.collective_compute(
                "AllToAll", ALU.bypass,
                replica_groups=[list(range(NCORES))],
                ins=[in_all[:].opt()], outs=[out_all[:].opt()],
            )

            # ---- collective window: rmsnorm stats + normalized x tiles
            rr_flats = []
            with tc.tile_pool(name="xsqp", bufs=1) as xsqp, \
                 tc.tile_pool(name="stm", bufs=2) as stm, \
                 tc.tile_pool(name="psS", bufs=2, space="PSUM") as psS:
                for i in range(SPC):
                    xt = xts[i]
                    xsq = xsqp.tile([HX, N], bf16)
                    for ch in range(2):
                        sl = slice(ch * (N // 2), (ch + 1) * (N // 2))
                        nc.vector.tensor_tensor(xsq[:, sl], xt[:, sl],
                                                xt[:, sl], ALU.mult)
                    pn_s = psS.tile([128, 32], fp32, name="pn_s")
                    for c in range(32):
                        nc.tensor.matmul(
                            pn_s[:, c:c + 1],
                            xsq[:, c * 128:(c + 1) * 128],
                            ones_col[:],
                            start=True, stop=True,
                        )
                    sq_m = stm.tile([128, 32], fp32, name="sq_m")
                    nc.scalar.activation(sq_m[:], pn_s[:], AF.Sqrt,
                                         bias=eps_t[:], scale=1.0 / HX)
                    rr = stm.tile([128, 32], fp32, name="rr")
                    nc.vector.reciprocal(rr[:], sq_m[:])
                    rr_t = psS.tile([32, 128], fp32, name="rr_t")
                    nc.tensor.transpose(rr_t[:], rr[:], ident[:])
                    rr_ts = stm.tile([32, 128], bf16, name="rr_ts")
                    nc.vector.tensor_copy(rr_ts[:], rr_t[:])
                    rr_flat = pers.tile([1, N], bf16, name=f"rr_flat{i}")
                    nc.scalar.dma_start(rr_flat[:], rr_ts[:])
                    rr_flats.append(rr_flat)

            xss = []
            for i in range(SPC):
                xs = xres.tile([HX, N], bf16, name=f"xs{i}")
                xss.append(xs)
            with tc.tile_pool(name="psB", bufs=2, space="PSUM") as psB, \
                 tc.tile_pool(name="rrs", bufs=2) as rrs:
                for i in range(SPC):
                    for h in range(2):
                        sl = slice(h * (N // 2), (h + 1) * (N // 2))
                        rrb = psB.tile([128, N // 2], fp32, name="rrb")
                        for q in range(4):
                            nc.tensor.matmul(
                                rrb[:, q * TS:(q + 1) * TS], ones_row[:],
                                rr_flats[i][0:1,
                                            h * (N // 2) + q * TS:
                                            h * (N // 2) + (q + 1) * TS],
                                start=True, stop=True)
                        if i < 2:
                            # scalar is idle in the window: cast first, so
                            # the DVE multiply runs at bf16 speed
                            rrc = rrs.tile([128, N // 2], bf16, name="rrc")
                            nc.scalar.activation(rrc[:], rrb[:], AF.Copy)
                            nc.vector.tensor_tensor(xss[i][:, sl],
                                                    xts[i][:, sl],
                                                    rrc[:], ALU.mult)
                        else:
                            nc.vector.tensor_tensor(xss[i][:, sl],
                                                    xts[i][:, sl],
                                                    rrb[:], ALU.mult)

            # ---- post-collective: gather per-sample weights, bias, norms
            with ExitStack() as es:
                def pool(name, bufs, space=None):
                    kw = {"space": space} if space else {}
                    return es.enter_context(
                        tc.tile_pool(name=name, bufs=bufs, **kw))
                p_fc1 = pool("fc1", 1)
                p_fc2 = pool("fc2", 1)
                p_fc1g = pool("fc1g", 1)
                p_rn = pool("rn", 1)
                p_tmp = pool("tmp", 2)
                p_sq = pool("sq", 2)
                p_h1 = pool("h1", 2)
                p_ob = pool("ob", 2)

                oa_sj = out_all[:, :].rearrange("(s j) c -> s j c", s=8)

                es_ps = ExitStack()
                p_pp = es_ps.enter_context(
                    tc.tile_pool(name="pp", bufs=1, space="PSUM"))
                p_rnb = es_ps.enter_context(
                    tc.tile_pool(name="rnb", bufs=2, space="PSUM"))

                # gathers: fc1 rows (sync ring), fc2 cols (scalar+gpsimd)
                fc1rs, fc2cs = [], []
                for j in range(SPC):
                    fc1r = p_fc1.tile([HX, F], fp8, name=f"fc1r{j}")
                    for sc in range(4):
                        nc.sync.dma_start(
                            fc1r[32 * sc:32 * (sc + 1), :],
                            oa_sj[sc, j, :].rearrange("(a f) -> a f", a=32),
                        )
                    fc2c = p_fc2.tile([128, 4 * HX], fp8,
                                      name=f"fc2c{j}")
                    for fb in range(4):
                        eng = nc.scalar if fb % 2 == 0 else nc.gpsimd
                        eng.dma_start(
                            fc2c[:, fb * HX:(fb + 1) * HX],
                            oa_sj[4 + fb, j, :].rearrange("(p d) -> p d",
                                                          p=128),
                        )
                    fc1rs.append(fc1r)
                    fc2cs.append(fc2c)

                # per-sample: bias, squares, norms, rsqrt, rank-1 fold.
                # Sample-major pnn columns (j*5 + {0..3} fc1 chunks, j*5+4
                # fc2) so each sample's chain completes independently.
                pnn = p_pp.tile([128, 20], fp32, name="pnn")
                fc1bs, fc2bs, fc1gs = [], [], []
                rn_sb = p_rn.tile([128, 20], fp32, name="rn_sb")
                for j in range(SPC):
                    fc1b = p_fc1.tile([HX, F], bf16, name=f"fc1b{j}")
                    nc.vector.tensor_tensor(fc1b[:], fc1rs[j][:], b1_t[:],
                                            ALU.add)
                    sq1 = p_sq.tile([HX, F], bf16, name="sq1")
                    nc.vector.tensor_tensor(sq1[:], fc1b[:], fc1b[:],
                                            ALU.mult)
                    for jc in range(4):
                        nc.tensor.matmul(
                            pnn[:, j * 5 + jc:j * 5 + jc + 1],
                            sq1[:, jc * 128:(jc + 1) * 128],
                            ones_col[:], start=True, stop=True)
                    fc2b = p_fc2.tile([128, 4 * HX], bf16, name=f"fc2b{j}")
                    nc.vector.tensor_tensor(fc2b[:], fc2cs[j][:], b2_t[:],
                                            ALU.add)
                    sq2 = p_sq.tile([128, 4 * HX], bf16, name="sq2")
                    nc.vector.tensor_tensor(sq2[:], fc2b[:], fc2b[:],
                                            ALU.mult)
                    for fb in range(4):
                        nc.tensor.matmul(
                            pnn[:, j * 5 + 4:j * 5 + 5],
                            sq2[:, fb * 128:(fb + 1) * 128],
                            ones_col[:], start=(fb == 0), stop=(fb == 3))
                    nn = p_tmp.tile([128, 5], fp32, name="nn")
                    nc.scalar.activation(nn[:], pnn[:, j * 5:j * 5 + 5],
                                         AF.Sqrt)
                    nc.vector.reciprocal(rn_sb[:, j * 5:j * 5 + 5], nn[:])
                    trn = p_pp.tile([4, 128], fp32, name="trn",
                                    uniquify=True)
                    nc.tensor.transpose(trn[:],
                                        rn_sb[:, j * 5:j * 5 + 4], ident[:])
                    rn_t = p_tmp.tile([4, 128], bf16, name="rn_t")
                    nc.vector.tensor_copy(rn_t[:], trn[:])
                    rn1f = p_rn.tile([1, F], bf16, name=f"rn1f{j}")
                    nc.gpsimd.dma_start(rn1f[:], rn_t[:])
                    rn1b = p_rnb.tile([HX, F], fp32, name="rn1b")
                    nc.tensor.matmul(rn1b[:], ones_row[:], rn1f[:],
                                     start=True, stop=True)
                    fc1g = p_fc1g.tile([HX, F], bf16, name=f"fc1g{j}")
                    nc.vector.scalar_tensor_tensor(
                        fc1g[:], fc1b[:], g_t[:], rn1b[:],
                        ALU.mult, ALU.mult)
                    fc1bs.append(fc1b)
                    fc2bs.append(fc2b)
                    fc1gs.append(fc1g)
                es_ps.close()

                # ---- main loop: software-pipelined (sample, token-block)
                with tc.tile_pool(name="ph1a", bufs=2,
                                  space="PSUM") as p_ph1a, \
                     tc.tile_pool(name="ph1b", bufs=1,
                                  space="PSUM") as p_ph1b, \
                     tc.tile_pool(name="ph2", bufs=2, space="PSUM") as p_ph2:
                    units = [(j, tb) for j in range(SPC) for tb in range(TB)]
                    prev = None

                    def bmm2_first(pv):
                        nc.tensor.matmul(pv["ph2"][:],
                                         fc2bs[pv["j"]][:, 0:HX],
                                         pv["h1a"][:, 0:TS],
                                         start=True, stop=False)
                        nc.tensor.matmul(pv["ph2"][:],
                                         fc2bs[pv["j"]][:, HX:2 * HX],
                                         pv["h1a"][:, TS:2 * TS],
                                         start=False, stop=False)

                    def bmm2_second(pv):
                        nc.tensor.matmul(pv["ph2"][:],
                                         fc2bs[pv["j"]][:, 2 * HX:3 * HX],
                                         pv["h1b"][:, 0:TS],
                                         start=False, stop=False)
                        nc.tensor.matmul(pv["ph2"][:],
                                         fc2bs[pv["j"]][:, 3 * HX:4 * HX],
                                         pv["h1b"][:, TS:2 * TS],
                                         start=False, stop=True)
                        ob = p_ob.tile([HX, TS], bf16)
                        nc.vector.scalar_tensor_tensor(
                            ob[:], pv["ph2"][:],
                            rn_sb[:, 5 * pv["j"] + 4:5 * pv["j"] + 5],
                            pv["xv"],
                            ALU.mult, ALU.add)
                        nc.sync.dma_start(
                            o_d[pv["j"], :,
                                pv["tb"] * TS:(pv["tb"] + 1) * TS],
                            ob[:])

                    for j, tb in units:
                        xv = xts[j][:, tb * TS:(tb + 1) * TS]
                        xsv = xss[j][:, tb * TS:(tb + 1) * TS]
                        fc1g = fc1gs[j]

                        ph1a = p_ph1a.tile([128, 2 * TS], fp32)
                        nc.tensor.matmul(ph1a[:, 0:TS], fc1g[:, 0:128], xsv,
                                         start=True, stop=True)
                        nc.tensor.matmul(ph1a[:, TS:2 * TS],
                                         fc1g[:, 128:256], xsv,
                                         start=True, stop=True)
                        h1a = p_h1.tile([128, 2 * TS], bf16)
                        nc.scalar.activation(h1a[:], ph1a[:], AF.Silu)

                        if prev is not None:
                            prev["ph2"] = p_ph2.tile([HX, TS], fp32,
                                                     name="ph2")
                            bmm2_first(prev)

                        ph1b = p_ph1b.tile([128, 2 * TS], fp32)
                        nc.tensor.matmul(ph1b[:, 0:TS], fc1g[:, 256:384],
                                         xsv, start=True, stop=True)
                        nc.tensor.matmul(ph1b[:, TS:2 * TS],
                                         fc1g[:, 384:512], xsv,
                                         start=True, stop=True)
                        h1b = p_h1.tile([128, 2 * TS], bf16)
                        nc.scalar.activation(h1b[:], ph1b[:], AF.Silu)

                        if prev is not None:
                            bmm2_second(prev)

                        prev = {"j": j, "tb": tb, "xv": xv,
                                "h1a": h1a, "h1b": h1b}

                    prev["ph2"] = p_ph2.tile([HX, TS], fp32, name="ph2")
                    bmm2_first(prev)
                    bmm2_second(prev)
    nc.compile()
    return nc


def _prep_inputs(x, s, W, b, g):
    e3m4 = ml_dtypes.float8_e3m4
    s_p = np.ascontiguousarray(
        s.T.reshape(8, 128, B).transpose(1, 0, 2).reshape(128, 8 * B)
    ).astype(np_bf16)
    g_p = np.ascontiguousarray(g.reshape(HX, 1)).astype(np.float32)
    b1 = np.ascontiguousarray(b[:HX * F].reshape(HX, F)).astype(np_bf16)
    b2 = np.ascontiguousarray(
        b[HX * F:].reshape(4, 128, 128).transpose(1, 0, 2).reshape(128, 512)
    ).astype(np_bf16)
    W8 = np.clip(W * WSCALE, -15.5, 15.5).astype(e3m4)
    in_maps = []
    for c in range(NCORES):
        Wc = W8[:, c * COLS:(c + 1) * COLS]
        # [HS, COLS] -> [NB, 128, 8*F] (block, partition, kt*F+j)
        Wc = np.ascontiguousarray(
            Wc.reshape(8, 128, NB, F).transpose(2, 1, 0, 3)
              .reshape(NB, 128, 8 * F))
        # group 4 consecutive blocks per partition row (16KB descriptors)
        Wc = np.ascontiguousarray(
            Wc.reshape(NG, 4, 128, 8 * F).transpose(0, 2, 1, 3)
              .reshape(NG, 128, 4 * 8 * F))
        xc = np.ascontiguousarray(
            x[SPC * c:SPC * (c + 1)].transpose(0, 2, 1)).astype(np_bf16)
        in_maps.append({"W": Wc, "x": xc, "s": s_p, "b1": b1, "b2": b2,
                        "g": g_p})
    return in_maps


def kernel(x, s, W, b, g):
    global LAST_EXEC_NS, _cached_nc
    x = np.asarray(x, dtype=np.float32)
    s = np.asarray(s, dtype=np.float32)
    W = np.asarray(W, dtype=np.float32)
    b = np.asarray(b, dtype=np.float32)
    g = np.asarray(g, dtype=np.float32)

    trace = os.environ.get("KERNEL_TRACE", "0") == "1"
    if trace:
        _ensure_axon_hooks()
    if _cached_nc is None:
        _cached_nc = _build()
    in_maps = _prep_inputs(x, s, W, b, g)
    res = run_bass_kernel_spmd(_cached_nc, in_maps, list(range(NCORES)),
                               trace=trace)
    LAST_EXEC_NS = res.exec_time_ns
    out = np.concatenate([res.results[c]["o"] for c in range(NCORES)], axis=0)
    return np.ascontiguousarray(
        out.transpose(0, 2, 1).astype(np.float32))
